# revision 6
# baseline (speedup 1.0000x reference)
"""Multi-head attention (B=2, S=2048, H=8, D=64) on 8 Trainium2 NeuronCores.

Sharding: (batch, head-pair) per core — core c handles batch b = c//4 and
heads h0 = 2*(c%4), h0+1.  Each core computes its 2 heads' attention scores
(written directly to the scores output) and a partial of the output
projection (ctx_heads @ Wo_heads); the host sums the 4 partials per batch
and adds bo.

Device kernel per core (all fp32):
  phase 1: transpose q/k/v tiles via PE, project to qh^T/kh^T [128, S] and
           vh [S, 128] layouts (1/sqrt(D) folded into Wq/bq on host —
           exact, power of 2).
  phase 2: per (sq-tile, head): scores = qh^T.T @ kh^T into PSUM;
           fused bias-add + row-max (tensor_tensor_reduce); mask fill via
           copy_predicated; exp with fused row-sum (ACT accum_out);
           normalize with reciprocal; store P; PE-transpose P chunks and
           accumulate ctx^T = vh.T @ P^T; finally ctx^T.T @ Wo -> partial out.
"""

import os
import sys

sys.path.insert(0, "/opt/trn_rl_repo")

import numpy as np

import concourse.bacc as bacc
import concourse.tile as tile
from concourse import mybir
from concourse.bass_utils import run_bass_kernel_spmd
from concourse.masks import make_identity

B, S, HIDDEN = 2, 2048, 512
H, D = 8, 64
HPC = 2                 # heads per core
C2 = HPC * D            # 128, the per-core head-channel dim
N_CORES = 8
ST = S // 128           # 16 sq tiles
JB = 4                  # sk blocks of 512
F32 = mybir.dt.float32
U8 = mybir.dt.uint8
MASK_FILL = -1e9

LAST_RESULTS = None     # test.py reads exec_time_ns from here


def _phase1(nc, tc, io, cs):
    """Project q/k/v into qh^T/kh^T [C2, S] and vh [S, C2] (as [128, ST, C2])."""
    for src, w_sb, kind in (
        (io["xq"], cs["wq_sb"], "q"),
        (io["xk"], cs["wk_sb"], "k"),
        (io["xv"], cs["wv_sb"], "v"),
    ):
        with (
            tc.tile_pool(name=f"p1x{kind}", bufs=3) as p1x,
            tc.tile_pool(name=f"p1xt{kind}", bufs=3) as p1xt,
            tc.tile_pool(name=f"p1tp{kind}", bufs=2, space="PSUM") as p1tp,
            tc.tile_pool(name=f"p1pp{kind}", bufs=2, space="PSUM") as p1pp,
        ):
            for t in range(ST):
                x_tile = p1x.tile([128, HIDDEN], F32, tag="x_tile")
                nc.sync.dma_start(out=x_tile, in_=src[t * 128 : (t + 1) * 128, :])
                xt_ps = p1tp.tile([128, HIDDEN], F32, tag="xt_ps")
                for c in range(4):
                    nc.tensor.transpose(
                        xt_ps[:, c * 128 : (c + 1) * 128],
                        x_tile[:, c * 128 : (c + 1) * 128],
                        cs["identity"],
                    )
                xt_sb = p1xt.tile([128, HIDDEN], F32, tag="xt_sb")
                nc.vector.tensor_copy(out=xt_sb, in_=xt_ps)

                proj_ps = p1pp.tile([128, 128], F32, tag="proj_ps")
                if kind in ("q", "k"):
                    for c in range(4):
                        nc.tensor.matmul(
                            proj_ps,
                            lhsT=w_sb[:, c, :],
                            rhs=xt_sb[:, c * 128 : (c + 1) * 128],
                            start=(c == 0),
                            stop=(c == 3),
                        )
                    dst = cs["qhT"] if kind == "q" else cs["khT"]
                    bias_ap = cs["bq_sb"] if kind == "q" else cs["bk_sb"]
                    nc.scalar.activation(
                        out=dst[:, t * 128 : (t + 1) * 128],
                        in_=proj_ps,
                        func=mybir.ActivationFunctionType.Identity,
                        bias=bias_ap,
                        scale=1.0,
                    )
                else:
                    for c in range(4):
                        nc.tensor.matmul(
                            proj_ps,
                            lhsT=xt_sb[:, c * 128 : (c + 1) * 128],
                            rhs=w_sb[:, c, :],
                            start=(c == 0),
                            stop=(c == 3),
                        )
                    nc.vector.tensor_tensor(
                        out=cs["v_sb"][:, t, :],
                        in0=proj_ps,
                        in1=cs["bvb_sb"],
                        op=mybir.AluOpType.add,
                    )


def _phase2(nc, tc, io, cs, kst, softmax_on, ctx_on):
    qhT, khT, v_sb = cs["qhT"], cs["khT"], cs["v_sb"]
    with (
        tc.tile_pool(name="maskp", bufs=2) as maskp,
        tc.tile_pool(name="biasp", bufs=3) as biasp,
        tc.tile_pool(name="sp", bufs=2) as sp,
        tc.tile_pool(name="pp", bufs=2) as pp,
        tc.tile_pool(name="pnp", bufs=3) as pnp,
        tc.tile_pool(name="ptsb", bufs=2) as ptsb,
        tc.tile_pool(name="ctx2", bufs=2) as ctx2,
        tc.tile_pool(name="posb", bufs=2) as posb,
        tc.tile_pool(name="small", bufs=4) as small,
        tc.tile_pool(name="scps", bufs=2, space="PSUM") as scps,
        tc.tile_pool(name="ptps", bufs=2, space="PSUM") as ptps,
        tc.tile_pool(name="ctxps", bufs=2, space="PSUM") as ctxps,
        tc.tile_pool(name="pops", bufs=1, space="PSUM") as pops,
    ):
        for t in range(kst):
            rs = t * 128
            mask_t = maskp.tile([128, S], U8, tag="mask_t")
            nc.sync.dma_start(out=mask_t, in_=io["maskb"][rs : rs + 128, :])
            ctxT2_sb = ctx2.tile([C2, 128], F32, tag="ctxT2")
            for h in range(HPC):
                hs = h * D
                bias_t = biasp.tile([128, S], F32, tag="bias_t")
                nc.sync.dma_start(out=bias_t, in_=io["bias2"][h, rs : rs + 128, :])
                s_sb = sp.tile([128, S], F32, tag="s_sb")
                for j in range(JB):
                    js = j * 512
                    sc_ps = scps.tile([128, 512], F32, tag="sc_ps")
                    nc.tensor.matmul(
                        sc_ps,
                        lhsT=qhT[hs : hs + D, rs : rs + 128],
                        rhs=khT[hs : hs + D, js : js + 512],
                        start=True,
                        stop=True,
                    )
                    nc.vector.tensor_tensor(
                        out=s_sb[:, js : js + 512],
                        in0=sc_ps,
                        in1=bias_t[:, js : js + 512],
                        op=mybir.AluOpType.add,
                    )
                    nc.vector.copy_predicated(
                        out=s_sb[:, js : js + 512],
                        mask=mask_t[:, js : js + 512],
                        data=cs["neg_tile"],
                    )
                pn = pnp.tile([128, S], F32, tag="pn")
                if softmax_on:
                    # No max-subtraction: softmax(x) is invariant to the
                    # shift, scores here are O(10) so exp can't overflow,
                    # and masked entries are exactly -1e9 -> exp == 0.
                    p_sb = pp.tile([128, S], F32, tag="p_sb")
                    esum = small.tile([128, 4], F32, tag="esum")
                    for j in range(JB):
                        js = j * 512
                        nc.scalar.activation(
                            out=p_sb[:, js : js + 512],
                            in_=s_sb[:, js : js + 512],
                            func=mybir.ActivationFunctionType.Exp,
                            bias=0.0,
                            scale=1.0,
                            accum_out=esum[:, j : j + 1],
                        )
                    rsum = small.tile([128, 1], F32, tag="rsum")
                    nc.vector.tensor_reduce(
                        out=rsum,
                        in_=esum,
                        axis=mybir.AxisListType.X,
                        op=mybir.AluOpType.add,
                    )
                    rinv = small.tile([128, 1], F32, tag="rinv")
                    nc.vector.reciprocal(out=rinv, in_=rsum)
                    for j in range(JB):
                        js = j * 512
                        nc.scalar.activation(
                            out=pn[:, js : js + 512],
                            in_=p_sb[:, js : js + 512],
                            func=mybir.ActivationFunctionType.Copy,
                            scale=rinv,
                        )
                else:
                    nc.vector.tensor_copy(out=pn, in_=s_sb)

                ctx_ps = ctxps.tile([D, 128], F32, tag="ctx_ps")
                if ctx_on:
                    for j in range(JB):
                        js = j * 512
                        pt_ps = ptps.tile([128, 512], F32, tag="pt_ps")
                        for cc in range(4):
                            nc.tensor.transpose(
                                pt_ps[:, cc * 128 : (cc + 1) * 128],
                                pn[:, js + cc * 128 : js + (cc + 1) * 128],
                                cs["identity"],
                            )
                        pt_sb = ptsb.tile([128, 512], F32, tag="pt_sb")
                        nc.any.tensor_copy(out=pt_sb, in_=pt_ps)
                        for cc in range(4):
                            g = j * 4 + cc
                            nc.tensor.matmul(
                                ctx_ps,
                                lhsT=v_sb[:, g, hs : hs + D],
                                rhs=pt_sb[:, cc * 128 : (cc + 1) * 128],
                                start=(g == 0),
                                stop=(g == 15),
                            )
                else:
                    nc.vector.memset(ctx_ps, 0.0)
                nc.sync.dma_start(out=io["scores2"][h, rs : rs + 128, :], in_=pn)
                nc.any.tensor_copy(out=ctxT2_sb[hs : hs + D, :], in_=ctx_ps)
            po_ps = pops.tile([128, HIDDEN], F32, tag="po_ps")
            nc.tensor.matmul(
                po_ps, lhsT=ctxT2_sb, rhs=cs["wo_sb"], start=True, stop=True
            )
            po_sb = posb.tile([128, HIDDEN], F32, tag="po_sb")
            nc.any.tensor_copy(out=po_sb, in_=po_ps)
            nc.sync.dma_start(out=io["pout"][rs : rs + 128, :], in_=po_sb)


def _build():
    # Bisect knobs (debug only): KPHASE1/KPHASE2/KSOFTMAX/KCTX in {0,1};
    # KST trims the number of sq-tiles in phase 2.
    p1_on = os.environ.get("KPHASE1", "1") == "1"
    p2_on = os.environ.get("KPHASE2", "1") == "1"
    softmax_on = os.environ.get("KSOFTMAX", "1") == "1"
    ctx_on = os.environ.get("KCTX", "1") == "1"
    kst = int(os.environ.get("KST", ST))

    nc = bacc.Bacc("TRN2", debug=False)

    io = {
        "xq": nc.declare_dram_parameter("xq", [S, HIDDEN], F32, isOutput=False),
        "xk": nc.declare_dram_parameter("xk", [S, HIDDEN], F32, isOutput=False),
        "xv": nc.declare_dram_parameter("xv", [S, HIDDEN], F32, isOutput=False),
        "bias2": nc.declare_dram_parameter("bias2", [HPC, S, S], F32, isOutput=False),
        "maskb": nc.declare_dram_parameter("maskb", [S, S], U8, isOutput=False),
        "wq": nc.declare_dram_parameter("wq", [HIDDEN, C2], F32, isOutput=False),
        "wk": nc.declare_dram_parameter("wk", [HIDDEN, C2], F32, isOutput=False),
        "wv": nc.declare_dram_parameter("wv", [HIDDEN, C2], F32, isOutput=False),
        "bq": nc.declare_dram_parameter("bq", [C2, 1], F32, isOutput=False),
        "bk": nc.declare_dram_parameter("bk", [C2, 1], F32, isOutput=False),
        "bvb": nc.declare_dram_parameter("bvb", [128, C2], F32, isOutput=False),
        "wo": nc.declare_dram_parameter("wo", [C2, HIDDEN], F32, isOutput=False),
        "scores2": nc.declare_dram_parameter("scores2", [HPC, S, S], F32, isOutput=True),
        "pout": nc.declare_dram_parameter("pout", [S, HIDDEN], F32, isOutput=True),
    }

    with tile.TileContext(nc) as tc:
        with (
            tc.tile_pool(name="consts", bufs=1) as consts,
            tc.tile_pool(name="persist", bufs=1) as persist,
        ):
            cs = {}
            cs["identity"] = consts.tile([128, 128], F32, name="identity", tag="identity")
            make_identity(nc, cs["identity"])
            cs["neg_tile"] = consts.tile([128, 512], F32, name="neg_tile", tag="neg_tile")
            nc.vector.memset(cs["neg_tile"], MASK_FILL)

            for wname in ("wq", "wk", "wv"):
                w_sb = consts.tile([128, 4, C2], F32, name=f"{wname}_sb", tag=f"{wname}_sb")
                nc.sync.dma_start(
                    out=w_sb, in_=io[wname][:, :].rearrange("(c p) m -> p c m", p=128)
                )
                cs[f"{wname}_sb"] = w_sb
            for bname in ("bq", "bk"):
                b_sb = consts.tile([C2, 1], F32, name=f"{bname}_sb", tag=f"{bname}_sb")
                nc.sync.dma_start(out=b_sb, in_=io[bname][:, :])
                cs[f"{bname}_sb"] = b_sb
            cs["bvb_sb"] = consts.tile([128, C2], F32, name="bvb_sb", tag="bvb_sb")
            nc.sync.dma_start(out=cs["bvb_sb"], in_=io["bvb"][:, :])
            cs["wo_sb"] = consts.tile([C2, HIDDEN], F32, name="wo_sb", tag="wo_sb")
            nc.sync.dma_start(out=cs["wo_sb"], in_=io["wo"][:, :])

            cs["qhT"] = persist.tile([C2, S], F32, name="qhT", tag="qhT")
            cs["khT"] = persist.tile([C2, S], F32, name="khT", tag="khT")
            cs["v_sb"] = persist.tile([128, ST, C2], F32, name="v_sb", tag="v_sb")
            if not p1_on:
                nc.vector.memset(cs["qhT"], 0.01)
                nc.vector.memset(cs["khT"], 0.01)
                nc.vector.memset(cs["v_sb"], 0.01)

            if p1_on:
                _phase1(nc, tc, io, cs)
            if p2_on:
                _phase2(nc, tc, io, cs, kst, softmax_on, ctx_on)
            else:
                # dump projections so the build has outputs
                for t in range(4):
                    tmp = persist.tile([C2, HIDDEN], F32, name=f"dump{t}", tag=f"dump{t}")
                    nc.vector.tensor_copy(out=tmp, in_=cs["qhT"][:, t * 512 : (t + 1) * 512])
                    nc.sync.dma_start(out=io["pout"][t * 128 : (t + 1) * 128, :], in_=tmp)

    nc.compile()
    return nc


_NC = None


def _get_nc():
    global _NC
    if _NC is None:
        _NC = _build()
    return _NC


def _in_maps(q, k, v, attn_bias, mask_u8, Wq, bq, Wk, bk, Wv, bv, Wo):
    scale = np.float32(D ** -0.5)          # 0.125, exact power of two
    maps = []
    for c in range(N_CORES):
        b = c // 4
        h0 = HPC * (c % 4)
        cslice = slice(h0 * D, (h0 + HPC) * D)
        maps.append(
            {
                "xq": np.ascontiguousarray(q[b]),
                "xk": np.ascontiguousarray(k[b]),
                "xv": np.ascontiguousarray(v[b]),
                "bias2": np.ascontiguousarray(attn_bias[b, h0 : h0 + HPC]),
                "maskb": mask_u8[b],
                "wq": np.ascontiguousarray(Wq[:, cslice]) * scale,
                "wk": np.ascontiguousarray(Wk[:, cslice]),
                "wv": np.ascontiguousarray(Wv[:, cslice]),
                "bq": (bq[cslice] * scale).reshape(C2, 1),
                "bk": bk[cslice].reshape(C2, 1).copy(),
                "bvb": np.broadcast_to(bv[cslice], (128, C2)).copy(),
                "wo": np.ascontiguousarray(Wo[cslice, :]),
            }
        )
    return maps


def kernel(q, k, v, attn_bias, attn_mask, Wq, bq, Wk, bk, Wv, bv, Wo, bo):
    global LAST_RESULTS
    q = np.asarray(q, np.float32)
    k = np.asarray(k, np.float32)
    v = np.asarray(v, np.float32)
    attn_bias = np.asarray(attn_bias, np.float32)
    mask_u8 = np.asarray(attn_mask).astype(np.uint8)
    Wq, Wk, Wv, Wo = (np.asarray(w, np.float32) for w in (Wq, Wk, Wv, Wo))
    bq, bk, bv, bo = (np.asarray(b_, np.float32) for b_ in (bq, bk, bv, bo))

    nc = _get_nc()
    res = run_bass_kernel_spmd(
        nc,
        _in_maps(q, k, v, attn_bias, mask_u8, Wq, bq, Wk, bk, Wv, bv, Wo),
        list(range(N_CORES)),
        trace=bool(os.environ.get("BASS_TRACE")),
    )
    LAST_RESULTS = res

    scores = np.empty((B, H, S, S), np.float32)
    out = np.zeros((B, S, HIDDEN), np.float32)
    for c in range(N_CORES):
        b = c // 4
        h0 = HPC * (c % 4)
        scores[b, h0 : h0 + HPC] = res.results[c]["scores2"]
        out[b] += res.results[c]["pout"]
    out += bo
    return out, scores


# revision 7
# speedup vs baseline: 1.1857x; 1.1857x over previous
"""Multi-head attention (B=2, S=2048, H=8, D=64) on 8 Trainium2 NeuronCores.

Sharding: (batch, head-pair) per core — core c handles batch b = c//4 and
heads h0 = 2*(c%4), h0+1.  Each core computes its 2 heads' attention scores
(written directly to the scores output) and a partial of the output
projection (ctx_heads @ Wo_heads); the host sums the 4 partials per batch
and adds bo.

Device kernel per core (all fp32):
  phase 1: transpose q/k/v tiles via PE, project to qh^T/kh^T [128, S] and
           vh [S, 128] layouts (1/sqrt(D) folded into Wq/bq on host —
           exact, power of 2).
  phase 2: per (sq-tile, head): scores = qh^T.T @ kh^T into PSUM;
           fused bias-add + row-max (tensor_tensor_reduce); mask fill via
           copy_predicated; exp with fused row-sum (ACT accum_out);
           normalize with reciprocal; store P; PE-transpose P chunks and
           accumulate ctx^T = vh.T @ P^T; finally ctx^T.T @ Wo -> partial out.
"""

import os
import sys

sys.path.insert(0, "/opt/trn_rl_repo")

import numpy as np

import concourse.bacc as bacc
import concourse.tile as tile
from concourse import mybir
from concourse.bass_utils import run_bass_kernel_spmd
from concourse.masks import make_identity

B, S, HIDDEN = 2, 2048, 512
H, D = 8, 64
HPC = 2                 # heads per core
C2 = HPC * D            # 128, the per-core head-channel dim
N_CORES = 8
ST = S // 128           # 16 sq tiles
JB = 4                  # sk blocks of 512
F32 = mybir.dt.float32
F16 = mybir.dt.float16
U8 = mybir.dt.uint8
MASK_FILL = -1e9

LAST_RESULTS = None     # test.py reads exec_time_ns from here


def _phase1(nc, tc, io, cs):
    """Project q/k/v into qh^T/kh^T [C2, S] and vh [S, C2] (as [128, ST, C2])."""
    for src, w_sb, kind in (
        (io["xq"], cs["wq_sb"], "q"),
        (io["xk"], cs["wk_sb"], "k"),
        (io["xv"], cs["wv_sb"], "v"),
    ):
        with (
            tc.tile_pool(name=f"p1x{kind}", bufs=3) as p1x,
            tc.tile_pool(name=f"p1xt{kind}", bufs=3) as p1xt,
            tc.tile_pool(name=f"p1tp{kind}", bufs=2, space="PSUM") as p1tp,
            tc.tile_pool(name=f"p1pp{kind}", bufs=2, space="PSUM") as p1pp,
        ):
            for t in range(ST):
                x_tile = p1x.tile([128, HIDDEN], F32, tag="x_tile")
                nc.sync.dma_start(out=x_tile, in_=src[t * 128 : (t + 1) * 128, :])
                xt_ps = p1tp.tile([128, HIDDEN], F32, tag="xt_ps")
                for c in range(4):
                    nc.tensor.transpose(
                        xt_ps[:, c * 128 : (c + 1) * 128],
                        x_tile[:, c * 128 : (c + 1) * 128],
                        cs["identity"],
                    )
                xt_sb = p1xt.tile([128, HIDDEN], F32, tag="xt_sb")
                nc.vector.tensor_copy(out=xt_sb, in_=xt_ps)

                proj_ps = p1pp.tile([128, 128], F32, tag="proj_ps")
                if kind in ("q", "k"):
                    for c in range(4):
                        nc.tensor.matmul(
                            proj_ps,
                            lhsT=w_sb[:, c, :],
                            rhs=xt_sb[:, c * 128 : (c + 1) * 128],
                            start=(c == 0),
                            stop=(c == 3),
                        )
                    dst = cs["qhT"] if kind == "q" else cs["khT"]
                    bias_ap = cs["bq_sb"] if kind == "q" else cs["bk_sb"]
                    nc.scalar.activation(
                        out=dst[:, t * 128 : (t + 1) * 128],
                        in_=proj_ps,
                        func=mybir.ActivationFunctionType.Identity,
                        bias=bias_ap,
                        scale=1.0,
                    )
                else:
                    for c in range(4):
                        nc.tensor.matmul(
                            proj_ps,
                            lhsT=xt_sb[:, c * 128 : (c + 1) * 128],
                            rhs=w_sb[:, c, :],
                            start=(c == 0),
                            stop=(c == 3),
                        )
                    nc.vector.tensor_tensor(
                        out=cs["v_sb"][:, t, :],
                        in0=proj_ps,
                        in1=cs["bvb_sb"],
                        op=mybir.AluOpType.add,
                    )


def _phase2(nc, tc, io, cs, kst, softmax_on, ctx_on):
    qhT, khT, v_sb = cs["qhT"], cs["khT"], cs["v_sb"]
    with (
        tc.tile_pool(name="maskp", bufs=2) as maskp,
        tc.tile_pool(name="biasp", bufs=3) as biasp,
        tc.tile_pool(name="sp", bufs=2) as sp,
        tc.tile_pool(name="pp", bufs=2) as pp,
        tc.tile_pool(name="pnp", bufs=3) as pnp,
        tc.tile_pool(name="ptsb", bufs=2) as ptsb,
        tc.tile_pool(name="ctx2", bufs=2) as ctx2,
        tc.tile_pool(name="posb", bufs=2) as posb,
        tc.tile_pool(name="small", bufs=4) as small,
        tc.tile_pool(name="scps", bufs=2, space="PSUM") as scps,
        tc.tile_pool(name="ptps", bufs=2, space="PSUM") as ptps,
        tc.tile_pool(name="ctxps", bufs=2, space="PSUM") as ctxps,
        tc.tile_pool(name="pops", bufs=1, space="PSUM") as pops,
    ):
        for t in range(kst):
            rs = t * 128
            mask_t = maskp.tile([128, S], U8, tag="mask_t")
            nc.sync.dma_start(out=mask_t, in_=io["maskb"][rs : rs + 128, :])
            ctxT2_sb = ctx2.tile([C2, 128], F32, tag="ctxT2")
            for h in range(HPC):
                hs = h * D
                bias_t = biasp.tile([128, S], F32, tag="bias_t")
                nc.sync.dma_start(out=bias_t, in_=io["bias2"][h, rs : rs + 128, :])
                s_sb = sp.tile([128, S], F32, tag="s_sb")
                for j in range(JB):
                    js = j * 512
                    sc_ps = scps.tile([128, 512], F32, tag="sc_ps")
                    nc.tensor.matmul(
                        sc_ps,
                        lhsT=qhT[hs : hs + D, rs : rs + 128],
                        rhs=khT[hs : hs + D, js : js + 512],
                        start=True,
                        stop=True,
                    )
                    nc.vector.tensor_tensor(
                        out=s_sb[:, js : js + 512],
                        in0=sc_ps,
                        in1=bias_t[:, js : js + 512],
                        op=mybir.AluOpType.add,
                    )
                    nc.vector.copy_predicated(
                        out=s_sb[:, js : js + 512],
                        mask=mask_t[:, js : js + 512],
                        data=cs["neg_tile"],
                    )
                pn = pnp.tile([128, S], F32, tag="pn")
                if softmax_on:
                    # No max-subtraction: softmax(x) is invariant to the
                    # shift, scores here are O(10) so exp can't overflow,
                    # and masked entries are exactly -1e9 -> exp == 0.
                    p_sb = pp.tile([128, S], F32, tag="p_sb")
                    esum = small.tile([128, 4], F32, tag="esum")
                    for j in range(JB):
                        js = j * 512
                        nc.scalar.activation(
                            out=p_sb[:, js : js + 512],
                            in_=s_sb[:, js : js + 512],
                            func=mybir.ActivationFunctionType.Exp,
                            bias=0.0,
                            scale=1.0,
                            accum_out=esum[:, j : j + 1],
                        )
                    rsum = small.tile([128, 1], F32, tag="rsum")
                    nc.vector.tensor_reduce(
                        out=rsum,
                        in_=esum,
                        axis=mybir.AxisListType.X,
                        op=mybir.AluOpType.add,
                    )
                    rinv = small.tile([128, 1], F32, tag="rinv")
                    nc.vector.reciprocal(out=rinv, in_=rsum)
                    for j in range(JB):
                        js = j * 512
                        nc.scalar.activation(
                            out=pn[:, js : js + 512],
                            in_=p_sb[:, js : js + 512],
                            func=mybir.ActivationFunctionType.Copy,
                            scale=rinv,
                        )
                else:
                    nc.vector.tensor_copy(out=pn, in_=s_sb)

                ctx_ps = ctxps.tile([D, 128], F32, tag="ctx_ps")
                if ctx_on:
                    for j in range(JB):
                        js = j * 512
                        pt_ps = ptps.tile([128, 512], F32, tag="pt_ps")
                        for cc in range(4):
                            nc.tensor.transpose(
                                pt_ps[:, cc * 128 : (cc + 1) * 128],
                                pn[:, js + cc * 128 : js + (cc + 1) * 128],
                                cs["identity"],
                            )
                        pt_sb = ptsb.tile([128, 512], F16, tag="pt_sb")
                        nc.any.tensor_copy(out=pt_sb, in_=pt_ps)
                        for cc in range(4):
                            g = j * 4 + cc
                            nc.tensor.matmul(
                                ctx_ps,
                                lhsT=v_sb[:, g, hs : hs + D],
                                rhs=pt_sb[:, cc * 128 : (cc + 1) * 128],
                                start=(g == 0),
                                stop=(g == 15),
                            )
                else:
                    nc.vector.memset(ctx_ps, 0.0)
                nc.sync.dma_start(out=io["scores2"][h, rs : rs + 128, :], in_=pn)
                nc.any.tensor_copy(out=ctxT2_sb[hs : hs + D, :], in_=ctx_ps)
            po_ps = pops.tile([128, HIDDEN], F32, tag="po_ps")
            nc.tensor.matmul(
                po_ps, lhsT=ctxT2_sb, rhs=cs["wo_sb"], start=True, stop=True
            )
            po_sb = posb.tile([128, HIDDEN], F32, tag="po_sb")
            nc.any.tensor_copy(out=po_sb, in_=po_ps)
            nc.sync.dma_start(out=io["pout"][rs : rs + 128, :], in_=po_sb)


def _build():
    # Bisect knobs (debug only): KPHASE1/KPHASE2/KSOFTMAX/KCTX in {0,1};
    # KST trims the number of sq-tiles in phase 2.
    p1_on = os.environ.get("KPHASE1", "1") == "1"
    p2_on = os.environ.get("KPHASE2", "1") == "1"
    softmax_on = os.environ.get("KSOFTMAX", "1") == "1"
    ctx_on = os.environ.get("KCTX", "1") == "1"
    kst = int(os.environ.get("KST", ST))

    nc = bacc.Bacc("TRN2", debug=False)

    io = {
        "xq": nc.declare_dram_parameter("xq", [S, HIDDEN], F32, isOutput=False),
        "xk": nc.declare_dram_parameter("xk", [S, HIDDEN], F32, isOutput=False),
        "xv": nc.declare_dram_parameter("xv", [S, HIDDEN], F32, isOutput=False),
        "bias2": nc.declare_dram_parameter("bias2", [HPC, S, S], F32, isOutput=False),
        "maskb": nc.declare_dram_parameter("maskb", [S, S], U8, isOutput=False),
        "wq": nc.declare_dram_parameter("wq", [HIDDEN, C2], F32, isOutput=False),
        "wk": nc.declare_dram_parameter("wk", [HIDDEN, C2], F32, isOutput=False),
        "wv": nc.declare_dram_parameter("wv", [HIDDEN, C2], F32, isOutput=False),
        "bq": nc.declare_dram_parameter("bq", [C2, 1], F32, isOutput=False),
        "bk": nc.declare_dram_parameter("bk", [C2, 1], F32, isOutput=False),
        "bvb": nc.declare_dram_parameter("bvb", [128, C2], F32, isOutput=False),
        "wo": nc.declare_dram_parameter("wo", [C2, HIDDEN], F32, isOutput=False),
        "scores2": nc.declare_dram_parameter("scores2", [HPC, S, S], F32, isOutput=True),
        "pout": nc.declare_dram_parameter("pout", [S, HIDDEN], F32, isOutput=True),
    }

    with tile.TileContext(nc) as tc:
        with (
            tc.tile_pool(name="consts", bufs=1) as consts,
            tc.tile_pool(name="persist", bufs=1) as persist,
        ):
            cs = {}
            cs["identity"] = consts.tile([128, 128], F32, name="identity", tag="identity")
            make_identity(nc, cs["identity"])
            cs["neg_tile"] = consts.tile([128, 512], F32, name="neg_tile", tag="neg_tile")
            nc.vector.memset(cs["neg_tile"], MASK_FILL)

            for wname in ("wq", "wk", "wv"):
                w_sb = consts.tile([128, 4, C2], F32, name=f"{wname}_sb", tag=f"{wname}_sb")
                nc.sync.dma_start(
                    out=w_sb, in_=io[wname][:, :].rearrange("(c p) m -> p c m", p=128)
                )
                cs[f"{wname}_sb"] = w_sb
            for bname in ("bq", "bk"):
                b_sb = consts.tile([C2, 1], F32, name=f"{bname}_sb", tag=f"{bname}_sb")
                nc.sync.dma_start(out=b_sb, in_=io[bname][:, :])
                cs[f"{bname}_sb"] = b_sb
            cs["bvb_sb"] = consts.tile([128, C2], F32, name="bvb_sb", tag="bvb_sb")
            nc.sync.dma_start(out=cs["bvb_sb"], in_=io["bvb"][:, :])
            cs["wo_sb"] = consts.tile([C2, HIDDEN], F32, name="wo_sb", tag="wo_sb")
            nc.sync.dma_start(out=cs["wo_sb"], in_=io["wo"][:, :])

            cs["qhT"] = persist.tile([C2, S], F16, name="qhT", tag="qhT")
            cs["khT"] = persist.tile([C2, S], F16, name="khT", tag="khT")
            cs["v_sb"] = persist.tile([128, ST, C2], F16, name="v_sb", tag="v_sb")
            if not p1_on:
                nc.vector.memset(cs["qhT"], 0.01)
                nc.vector.memset(cs["khT"], 0.01)
                nc.vector.memset(cs["v_sb"], 0.01)

            if p1_on:
                _phase1(nc, tc, io, cs)
            if p2_on:
                _phase2(nc, tc, io, cs, kst, softmax_on, ctx_on)
            else:
                # dump projections so the build has outputs
                for t in range(4):
                    tmp = persist.tile([C2, HIDDEN], F32, name=f"dump{t}", tag=f"dump{t}")
                    nc.vector.tensor_copy(out=tmp, in_=cs["qhT"][:, t * 512 : (t + 1) * 512])
                    nc.sync.dma_start(out=io["pout"][t * 128 : (t + 1) * 128, :], in_=tmp)

    nc.compile()
    return nc


_NC = None


def _get_nc():
    global _NC
    if _NC is None:
        _NC = _build()
    return _NC


def _in_maps(q, k, v, attn_bias, mask_u8, Wq, bq, Wk, bk, Wv, bv, Wo):
    scale = np.float32(D ** -0.5)          # 0.125, exact power of two
    maps = []
    for c in range(N_CORES):
        b = c // 4
        h0 = HPC * (c % 4)
        cslice = slice(h0 * D, (h0 + HPC) * D)
        maps.append(
            {
                "xq": np.ascontiguousarray(q[b]),
                "xk": np.ascontiguousarray(k[b]),
                "xv": np.ascontiguousarray(v[b]),
                "bias2": np.ascontiguousarray(attn_bias[b, h0 : h0 + HPC]),
                "maskb": mask_u8[b],
                "wq": np.ascontiguousarray(Wq[:, cslice]) * scale,
                "wk": np.ascontiguousarray(Wk[:, cslice]),
                "wv": np.ascontiguousarray(Wv[:, cslice]),
                "bq": (bq[cslice] * scale).reshape(C2, 1),
                "bk": bk[cslice].reshape(C2, 1).copy(),
                "bvb": np.broadcast_to(bv[cslice], (128, C2)).copy(),
                "wo": np.ascontiguousarray(Wo[cslice, :]),
            }
        )
    return maps


def kernel(q, k, v, attn_bias, attn_mask, Wq, bq, Wk, bk, Wv, bv, Wo, bo):
    global LAST_RESULTS
    q = np.asarray(q, np.float32)
    k = np.asarray(k, np.float32)
    v = np.asarray(v, np.float32)
    attn_bias = np.asarray(attn_bias, np.float32)
    mask_u8 = np.asarray(attn_mask).astype(np.uint8)
    Wq, Wk, Wv, Wo = (np.asarray(w, np.float32) for w in (Wq, Wk, Wv, Wo))
    bq, bk, bv, bo = (np.asarray(b_, np.float32) for b_ in (bq, bk, bv, bo))

    nc = _get_nc()
    res = run_bass_kernel_spmd(
        nc,
        _in_maps(q, k, v, attn_bias, mask_u8, Wq, bq, Wk, bk, Wv, bv, Wo),
        list(range(N_CORES)),
        trace=bool(os.environ.get("BASS_TRACE")),
    )
    LAST_RESULTS = res

    scores = np.empty((B, H, S, S), np.float32)
    out = np.zeros((B, S, HIDDEN), np.float32)
    for c in range(N_CORES):
        b = c // 4
        h0 = HPC * (c % 4)
        scores[b, h0 : h0 + HPC] = res.results[c]["scores2"]
        out[b] += res.results[c]["pout"]
    out += bo
    return out, scores


# revision 8
# speedup vs baseline: 1.3029x; 1.0988x over previous
"""Multi-head attention (B=2, S=2048, H=8, D=64) on 8 Trainium2 NeuronCores.

Sharding: (batch, head-pair) per core — core c handles batch b = c//4 and
heads h0 = 2*(c%4), h0+1.  Each core computes its 2 heads' attention scores
(written directly to the scores output) and a partial of the output
projection (ctx_heads @ Wo_heads); the host sums the 4 partials per batch
and adds bo.

Device kernel per core (all fp32):
  phase 1: transpose q/k/v tiles via PE, project to qh^T/kh^T [128, S] and
           vh [S, 128] layouts (1/sqrt(D) folded into Wq/bq on host —
           exact, power of 2).
  phase 2: per (sq-tile, head): scores = qh^T.T @ kh^T into PSUM;
           fused bias-add + row-max (tensor_tensor_reduce); mask fill via
           copy_predicated; exp with fused row-sum (ACT accum_out);
           normalize with reciprocal; store P; PE-transpose P chunks and
           accumulate ctx^T = vh.T @ P^T; finally ctx^T.T @ Wo -> partial out.
"""

import os
import sys

sys.path.insert(0, "/opt/trn_rl_repo")

import numpy as np

import concourse.bacc as bacc
import concourse.tile as tile
from concourse import mybir
from concourse.bass_utils import run_bass_kernel_spmd
from concourse.masks import make_identity

B, S, HIDDEN = 2, 2048, 512
H, D = 8, 64
HPC = 2                 # heads per core
C2 = HPC * D            # 128, the per-core head-channel dim
N_CORES = 8
ST = S // 128           # 16 sq tiles
JB = 4                  # sk blocks of 512
F32 = mybir.dt.float32
F16 = mybir.dt.float16
U8 = mybir.dt.uint8
MASK_FILL = -1e9

LAST_RESULTS = None     # test.py reads exec_time_ns from here


def _phase1(nc, tc, io, cs):
    """Project f16 q/k/v (DMA-transposed loads) into qh^T/kh^T [C2, S] f16
    and vh [S, C2] f16 (as [128, ST, C2], via PE-transpose of vh^T)."""
    for src, w_sb, kind in (
        (io["xq"], cs["wq_sb"], "q"),
        (io["xk"], cs["wk_sb"], "k"),
        (io["xv"], cs["wv_sb"], "v"),
    ):
        with (
            tc.tile_pool(name=f"p1xt{kind}", bufs=1) as p1xt,
            tc.tile_pool(name=f"p1pp{kind}", bufs=2, space="PSUM") as p1pp,
            tc.tile_pool(name=f"p1vt{kind}", bufs=2, space="PSUM") as p1vt,
        ):
            xt = p1xt.tile([128, 4, S], F16, name=f"xt{kind}", tag="xt")
            for c in range(4):
                nc.sync.dma_start_transpose(
                    out=xt[:, c, :], in_=src[:, c * 128 : (c + 1) * 128]
                )
            if kind == "q":
                dst, bias_ap = cs["qhT"], cs["bq_sb"]
            elif kind == "k":
                dst, bias_ap = cs["khT"], cs["bk_sb"]
            else:
                dst = p1xt.tile([C2, S], F16, name="vhT", tag="vhT")
                bias_ap = cs["bv_sb"]
            for blk in range(4):
                bs = blk * 512
                proj_ps = p1pp.tile([128, 512], F32, tag="proj_ps")
                for c in range(4):
                    nc.tensor.matmul(
                        proj_ps,
                        lhsT=w_sb[:, c, :],
                        rhs=xt[:, c, bs : bs + 512],
                        start=(c == 0),
                        stop=(c == 3),
                    )
                nc.scalar.activation(
                    out=dst[:, bs : bs + 512],
                    in_=proj_ps,
                    func=mybir.ActivationFunctionType.Identity,
                    bias=bias_ap,
                    scale=1.0,
                )
            if kind == "v":
                for g in range(ST):
                    vt_ps = p1vt.tile([128, 128], F16, tag="vt_ps")
                    nc.tensor.transpose(
                        vt_ps, dst[:, g * 128 : (g + 1) * 128], cs["identity16"]
                    )
                    nc.any.tensor_copy(out=cs["v_sb"][:, g, :], in_=vt_ps)


def _phase2(nc, tc, io, cs, kst, softmax_on, ctx_on):
    qhT, khT, v_sb = cs["qhT"], cs["khT"], cs["v_sb"]
    with (
        tc.tile_pool(name="maskp", bufs=2) as maskp,
        tc.tile_pool(name="biasp", bufs=3) as biasp,
        tc.tile_pool(name="sp", bufs=2) as sp,
        tc.tile_pool(name="pp", bufs=2) as pp,
        tc.tile_pool(name="pnp", bufs=3) as pnp,
        tc.tile_pool(name="ptsb", bufs=2) as ptsb,
        tc.tile_pool(name="ctx2", bufs=2) as ctx2,
        tc.tile_pool(name="posb", bufs=2) as posb,
        tc.tile_pool(name="small", bufs=4) as small,
        tc.tile_pool(name="scps", bufs=2, space="PSUM") as scps,
        tc.tile_pool(name="ptps", bufs=2, space="PSUM") as ptps,
        tc.tile_pool(name="ctxps", bufs=2, space="PSUM") as ctxps,
        tc.tile_pool(name="pops", bufs=1, space="PSUM") as pops,
    ):
        for t in range(kst):
            rs = t * 128
            mask_t = maskp.tile([128, S], U8, tag="mask_t")
            nc.sync.dma_start(out=mask_t, in_=io["maskb"][rs : rs + 128, :])
            ctxT2_sb = ctx2.tile([C2, 128], F32, tag="ctxT2")
            for h in range(HPC):
                hs = h * D
                bias_t = biasp.tile([128, S], F32, tag="bias_t")
                nc.sync.dma_start(out=bias_t, in_=io["bias2"][h, rs : rs + 128, :])
                s_sb = sp.tile([128, S], F32, tag="s_sb")
                for j in range(JB):
                    js = j * 512
                    sc_ps = scps.tile([128, 512], F32, tag="sc_ps")
                    nc.tensor.matmul(
                        sc_ps,
                        lhsT=qhT[hs : hs + D, rs : rs + 128],
                        rhs=khT[hs : hs + D, js : js + 512],
                        start=True,
                        stop=True,
                    )
                    nc.vector.tensor_tensor(
                        out=s_sb[:, js : js + 512],
                        in0=sc_ps,
                        in1=bias_t[:, js : js + 512],
                        op=mybir.AluOpType.add,
                    )
                    nc.vector.copy_predicated(
                        out=s_sb[:, js : js + 512],
                        mask=mask_t[:, js : js + 512],
                        data=cs["neg_tile"],
                    )
                pn = pnp.tile([128, S], F32, tag="pn")
                if softmax_on:
                    # No max-subtraction: softmax(x) is invariant to the
                    # shift, scores here are O(10) so exp can't overflow,
                    # and masked entries are exactly -1e9 -> exp == 0.
                    p_sb = pp.tile([128, S], F32, tag="p_sb")
                    esum = small.tile([128, 4], F32, tag="esum")
                    for j in range(JB):
                        js = j * 512
                        nc.scalar.activation(
                            out=p_sb[:, js : js + 512],
                            in_=s_sb[:, js : js + 512],
                            func=mybir.ActivationFunctionType.Exp,
                            bias=0.0,
                            scale=1.0,
                            accum_out=esum[:, j : j + 1],
                        )
                    rsum = small.tile([128, 1], F32, tag="rsum")
                    nc.vector.tensor_reduce(
                        out=rsum,
                        in_=esum,
                        axis=mybir.AxisListType.X,
                        op=mybir.AluOpType.add,
                    )
                    rinv = small.tile([128, 1], F32, tag="rinv")
                    nc.vector.reciprocal(out=rinv, in_=rsum)
                    for j in range(JB):
                        js = j * 512
                        nc.scalar.activation(
                            out=pn[:, js : js + 512],
                            in_=p_sb[:, js : js + 512],
                            func=mybir.ActivationFunctionType.Copy,
                            scale=rinv,
                        )
                else:
                    nc.vector.tensor_copy(out=pn, in_=s_sb)

                ctx_ps = ctxps.tile([D, 128], F32, tag="ctx_ps")
                if ctx_on:
                    for j in range(JB):
                        js = j * 512
                        pt_ps = ptps.tile([128, 512], F32, tag="pt_ps")
                        for cc in range(4):
                            nc.tensor.transpose(
                                pt_ps[:, cc * 128 : (cc + 1) * 128],
                                pn[:, js + cc * 128 : js + (cc + 1) * 128],
                                cs["identity"],
                            )
                        pt_sb = ptsb.tile([128, 512], F16, tag="pt_sb")
                        nc.any.tensor_copy(out=pt_sb, in_=pt_ps)
                        for cc in range(4):
                            g = j * 4 + cc
                            nc.tensor.matmul(
                                ctx_ps,
                                lhsT=v_sb[:, g, hs : hs + D],
                                rhs=pt_sb[:, cc * 128 : (cc + 1) * 128],
                                start=(g == 0),
                                stop=(g == 15),
                            )
                else:
                    nc.vector.memset(ctx_ps, 0.0)
                nc.sync.dma_start(out=io["scores2"][h, rs : rs + 128, :], in_=pn)
                nc.any.tensor_copy(out=ctxT2_sb[hs : hs + D, :], in_=ctx_ps)
            po_ps = pops.tile([128, HIDDEN], F32, tag="po_ps")
            nc.tensor.matmul(
                po_ps, lhsT=ctxT2_sb, rhs=cs["wo_sb"], start=True, stop=True
            )
            po_sb = posb.tile([128, HIDDEN], F32, tag="po_sb")
            nc.any.tensor_copy(out=po_sb, in_=po_ps)
            nc.sync.dma_start(out=io["pout"][rs : rs + 128, :], in_=po_sb)


def _build():
    # Bisect knobs (debug only): KPHASE1/KPHASE2/KSOFTMAX/KCTX in {0,1};
    # KST trims the number of sq-tiles in phase 2.
    p1_on = os.environ.get("KPHASE1", "1") == "1"
    p2_on = os.environ.get("KPHASE2", "1") == "1"
    softmax_on = os.environ.get("KSOFTMAX", "1") == "1"
    ctx_on = os.environ.get("KCTX", "1") == "1"
    kst = int(os.environ.get("KST", ST))

    nc = bacc.Bacc("TRN2", debug=False)

    io = {
        "xq": nc.declare_dram_parameter("xq", [S, HIDDEN], F16, isOutput=False),
        "xk": nc.declare_dram_parameter("xk", [S, HIDDEN], F16, isOutput=False),
        "xv": nc.declare_dram_parameter("xv", [S, HIDDEN], F16, isOutput=False),
        "bias2": nc.declare_dram_parameter("bias2", [HPC, S, S], F32, isOutput=False),
        "maskb": nc.declare_dram_parameter("maskb", [S, S], U8, isOutput=False),
        "wq": nc.declare_dram_parameter("wq", [HIDDEN, C2], F16, isOutput=False),
        "wk": nc.declare_dram_parameter("wk", [HIDDEN, C2], F16, isOutput=False),
        "wv": nc.declare_dram_parameter("wv", [HIDDEN, C2], F16, isOutput=False),
        "bq": nc.declare_dram_parameter("bq", [C2, 1], F32, isOutput=False),
        "bk": nc.declare_dram_parameter("bk", [C2, 1], F32, isOutput=False),
        "bv": nc.declare_dram_parameter("bv", [C2, 1], F32, isOutput=False),
        "wo": nc.declare_dram_parameter("wo", [C2, HIDDEN], F32, isOutput=False),
        "scores2": nc.declare_dram_parameter("scores2", [HPC, S, S], F32, isOutput=True),
        "pout": nc.declare_dram_parameter("pout", [S, HIDDEN], F32, isOutput=True),
    }

    with tile.TileContext(nc) as tc:
        with (
            tc.tile_pool(name="consts", bufs=1) as consts,
            tc.tile_pool(name="persist", bufs=1) as persist,
        ):
            cs = {}
            cs["identity"] = consts.tile([128, 128], F32, name="identity", tag="identity")
            make_identity(nc, cs["identity"])
            cs["neg_tile"] = consts.tile([128, 512], F32, name="neg_tile", tag="neg_tile")
            nc.vector.memset(cs["neg_tile"], MASK_FILL)
            cs["identity16"] = consts.tile([128, 128], F16, name="identity16", tag="identity16")
            make_identity(nc, cs["identity16"])

            for wname in ("wq", "wk", "wv"):
                w_sb = consts.tile([128, 4, C2], F16, name=f"{wname}_sb", tag=f"{wname}_sb")
                nc.sync.dma_start(
                    out=w_sb, in_=io[wname][:, :].rearrange("(c p) m -> p c m", p=128)
                )
                cs[f"{wname}_sb"] = w_sb
            for bname in ("bq", "bk", "bv"):
                b_sb = consts.tile([C2, 1], F32, name=f"{bname}_sb", tag=f"{bname}_sb")
                nc.sync.dma_start(out=b_sb, in_=io[bname][:, :])
                cs[f"{bname}_sb"] = b_sb
            cs["wo_sb"] = consts.tile([C2, HIDDEN], F32, name="wo_sb", tag="wo_sb")
            nc.sync.dma_start(out=cs["wo_sb"], in_=io["wo"][:, :])

            cs["qhT"] = persist.tile([C2, S], F16, name="qhT", tag="qhT")
            cs["khT"] = persist.tile([C2, S], F16, name="khT", tag="khT")
            cs["v_sb"] = persist.tile([128, ST, C2], F16, name="v_sb", tag="v_sb")
            if not p1_on:
                nc.vector.memset(cs["qhT"], 0.01)
                nc.vector.memset(cs["khT"], 0.01)
                nc.vector.memset(cs["v_sb"], 0.01)

            if p1_on:
                _phase1(nc, tc, io, cs)
            if p2_on:
                _phase2(nc, tc, io, cs, kst, softmax_on, ctx_on)
            else:
                # dump projections so the build has outputs
                for t in range(4):
                    tmp = persist.tile([C2, HIDDEN], F32, name=f"dump{t}", tag=f"dump{t}")
                    nc.vector.tensor_copy(out=tmp, in_=cs["qhT"][:, t * 512 : (t + 1) * 512])
                    nc.sync.dma_start(out=io["pout"][t * 128 : (t + 1) * 128, :], in_=tmp)

    nc.compile()
    return nc


_NC = None


def _get_nc():
    global _NC
    if _NC is None:
        _NC = _build()
    return _NC


def _in_maps(q, k, v, attn_bias, mask_u8, Wq, bq, Wk, bk, Wv, bv, Wo):
    scale = np.float32(D ** -0.5)          # 0.125, exact power of two
    maps = []
    for c in range(N_CORES):
        b = c // 4
        h0 = HPC * (c % 4)
        cslice = slice(h0 * D, (h0 + HPC) * D)
        maps.append(
            {
                "xq": q[b].astype(np.float16),
                "xk": k[b].astype(np.float16),
                "xv": v[b].astype(np.float16),
                "bias2": np.ascontiguousarray(attn_bias[b, h0 : h0 + HPC]),
                "maskb": mask_u8[b],
                "wq": (Wq[:, cslice] * scale).astype(np.float16),
                "wk": Wk[:, cslice].astype(np.float16),
                "wv": Wv[:, cslice].astype(np.float16),
                "bq": (bq[cslice] * scale).reshape(C2, 1),
                "bk": bk[cslice].reshape(C2, 1).copy(),
                "bv": bv[cslice].reshape(C2, 1).copy(),
                "wo": np.ascontiguousarray(Wo[cslice, :]),
            }
        )
    return maps


def kernel(q, k, v, attn_bias, attn_mask, Wq, bq, Wk, bk, Wv, bv, Wo, bo):
    global LAST_RESULTS
    q = np.asarray(q, np.float32)
    k = np.asarray(k, np.float32)
    v = np.asarray(v, np.float32)
    attn_bias = np.asarray(attn_bias, np.float32)
    mask_u8 = np.asarray(attn_mask).astype(np.uint8)
    Wq, Wk, Wv, Wo = (np.asarray(w, np.float32) for w in (Wq, Wk, Wv, Wo))
    bq, bk, bv, bo = (np.asarray(b_, np.float32) for b_ in (bq, bk, bv, bo))

    nc = _get_nc()
    res = run_bass_kernel_spmd(
        nc,
        _in_maps(q, k, v, attn_bias, mask_u8, Wq, bq, Wk, bk, Wv, bv, Wo),
        list(range(N_CORES)),
        trace=bool(os.environ.get("BASS_TRACE")),
    )
    LAST_RESULTS = res

    scores = np.empty((B, H, S, S), np.float32)
    out = np.zeros((B, S, HIDDEN), np.float32)
    for c in range(N_CORES):
        b = c // 4
        h0 = HPC * (c % 4)
        scores[b, h0 : h0 + HPC] = res.results[c]["scores2"]
        out[b] += res.results[c]["pout"]
    out += bo
    return out, scores
